# revision 63
# baseline (speedup 1.0000x reference)
"""Causal single-head attention on 8 trn2 NeuronCores.

Problem: x [4, 2048, 1024] f32; Wq/Wk/Wv [1024, 1024] f32.
  q,k,v = x@W*; scores = q@k^T (causal masked, scaled 1/sqrt(1024));
  out = softmax(scores) @ v.

Key algebra: scores = (x@Wq)(x@Wk)^T = x @ G @ x^T with G = Wq@Wk^T
precomputed on host. This removes the Q AND K projections from the
device: one GEMM t = x_q @ G replaces both, and the scores stationary
operand becomes raw x^T (resident in SBUF anyway). V is never built
either: out = ((attn @ x) @ Wv) / rowsum.

Sharding: 8 cores = 4 batches x 2 query-parities. Core c: batch c//2,
parity h=c%2 owns the 256-row query cols {0,3,4,7} (h=0) or {1,2,5,6}
(h=1) -- both parities see causal extents {1,2,3,4} (in 512-key cols),
so one SPMD program fits all cores; per-core causal masks ride in as
data and cover the <=256 keys of block padding per col.

Precision (rel_inf 1.24e-2 / rel_L2 1.76e-2 vs 2e-2 gate, host-simulated
exactly and bit-deterministic on hw):
  The scores matmuls run fp8 e4m3 with DoubleRow perf mode (256-deep
  contraction per instruction, 2x PE throughput): x^T (stationary) and
  t (moving) quantize to fp8 at the PSUM->SBUF copy. The t-projection
  runs HALF its contraction (d-chunks 0-3) in fp8-DR, half in bf16 --
  the error-vs-speed knee that keeps both error norms under the gate.
  The whole v path (EXPS, x, Wv) stays bf16 -- fp8 there pushes rel_L2
  past the gate. Matmul moving rate is dtype-flat (~0.5 ns/col
  measured), so fp8 pays off exactly where DoubleRow halves
  instruction count.

Schedule notes (per trace analysis): masks/x/Wv all SBUF-resident, bulk
prefetch paced on the sync DMA queue, latency-critical small DMAs on the
gpsimd queue (desc-gen ~0.6-1us serialized per queue); scores for the
next query col run before this col's TT/out so every rowsum DRAM
roundtrip hides under a full TT+out block; the fully-masked lower-half
of each col's last key block is skipped with half-width matmuls.

Per-core kernel:
  phase 1:  tT[e,qn] = G-chunks^T . xTq      (bf16, 16 groups x 8)
  phase 2, per local query col (256 wide):
    scoresT[kn,qn] = xT-chunks^T . tT        (fp8 DR, 4 matmuls/block)
    expT = exp(scoresT/32) -> bf16  (ACT; no max-sub: |s|/32 < ~3)
    last-4 kn-block tiles *= mask            (host-provided, DVE)
    rowsum[1,qn] = ones^T . expT             (FD512 pairs, bf16)
      -> DRAM roundtrip transpose -> [qn,1] -> reciprocal (off crit path)
    TT[d,qn]   = x-chunks^T . expT           (XK resident bf16)
    out[qn,e]  = TT-chunks^T . Wv            (bf16)
    out *= 1/rowsum (per-partition scalar), DMA out f32.

kernel() is self-contained: shards on host, runs via run_bass_kernel_spmd
on cores 0-7, reassembles the full [4, 2048, 1024] f32 output.
"""

import numpy as np
import ml_dtypes
from contextlib import ExitStack

import concourse.bass as bass
import concourse.mybir as mybir
import concourse.tile as tile
from concourse import bacc
from concourse.bass_utils import run_bass_kernel_spmd

P = 128
D = 1024          # d_in == d_out
NSEQ = 2048
NCOL = 512        # projection moving width / key-col unit
QW = 256          # query col width in phase 2
DB = D // P       # 8 d blocks
EB = D // P       # 8 e blocks
# local col order (2,4,3,1) by extent: tiny col ends the kernel (short tail)
EXT = (2, 4, 3, 1)           # causal extent per local q col, in 512-key cols
QCOLS = {0: (3, 7, 4, 0), 1: (2, 6, 5, 1)}  # parity -> global 256-q-cols

_f32 = mybir.dt.float32
_bf16 = mybir.dt.bfloat16
_fp8 = mybir.dt.float8e4
_DR = mybir.MatmulPerfMode.DoubleRow

_BUILD_CACHE = {}


def _build():
    if "nc" in _BUILD_CACHE:
        return _BUILD_CACHE["nc"]

    nc = bacc.Bacc("TRN2", target_bir_lowering=False, debug=False, num_devices=8)
    # host-pretiled inputs; every DMA below is contiguous per partition
    # xt[p, ic, db, n]   = x^T[db*128+p, ic*512+n]        (fp8, scores stat.)
    # xtq[p, jc, db, n]  = gathered-q x^T[db*128+p, jc*512+n]  (bf16)
    # xk[p, kb, db, m]   = x[kb*128+p, db*128+m]          (bf16, TT stat.)
    # g[p, eb, db, m]    = G[db*128+p, eb*128+m]          (bf16)
    # wv[p, db, ec, n]   = Wv[db*128+p, ec*512+n]         (bf16)
    xt = nc.dram_tensor("xt", [P, 4, DB, NCOL], _fp8, kind="ExternalInput").ap()
    # phase-1 contraction split: d-chunks 0-3 in fp8 (DoubleRow), 4-7 bf16
    xq8 = nc.dram_tensor("xq8", [P, 2, 4, NCOL], _fp8, kind="ExternalInput").ap()
    xq16 = nc.dram_tensor("xq16", [P, 2, 4, NCOL], _bf16, kind="ExternalInput").ap()
    xk = nc.dram_tensor("xk", [P, 16, DB, P], _bf16, kind="ExternalInput").ap()
    g8 = nc.dram_tensor("g8", [P, EB, 4, P], _fp8, kind="ExternalInput").ap()
    g16 = nc.dram_tensor("g16", [P, EB, 4, P], _bf16, kind="ExternalInput").ap()
    wv = nc.dram_tensor("wv", [P, DB, 2, NCOL], _bf16, kind="ExternalInput").ap()
    msk = nc.dram_tensor("msk", [P, 16, QW], _bf16, kind="ExternalInput").ap()
    onesd = nc.dram_tensor("ones", [P, 1], _bf16, kind="ExternalInput").ap()
    out = nc.dram_tensor("out", [1024, D], _f32, kind="ExternalOutput").ap()

    # G is host-prescaled by 8 (keeps the fp8 half out of e4m3 subnormals);
    # scores arrive 8x hot, folded into the exp scale
    scale = float(1.0 / np.sqrt(D) / 8.0)

    with tile.TileContext(nc) as tc, ExitStack() as ctx:
        pers = ctx.enter_context(tc.tile_pool(name="pers", bufs=1))
        XT = pers.tile([P, 4, DB, NCOL], _fp8)       # 16 KB/part
        XK = pers.tile([P, 16, DB, P], _bf16)        # 32
        # one tT tile per query-col-pair: scores for cols 0/1 then only
        # depend on the first half of phase 1 (no whole-tile WAR stall)
        TQs = [pers.tile([P, EB, NCOL], _fp8, name=f"tq{j}") for j in range(2)]
        WV = pers.tile([P, DB, 2, NCOL], _bf16)      # 16
        EXPS = pers.tile([P, 32, QW], _bf16)         # 16
        TTs = pers.tile([P, DB, QW], _bf16)          # 4
        MSK = pers.tile([P, 16, QW], _bf16)          # 8
        ONES = pers.tile([P, 1], _bf16)

        # ---- phase 1: tT projection (t = x_q @ 8G), half fp8-DR half bf16 ----
        with ExitStack() as p1:
            xtqpool = p1.enter_context(tc.tile_pool(name="xtqp", bufs=1))
            XTQ8 = xtqpool.tile([P, 2, 4, NCOL], _fp8)    # 4
            XTQ16 = xtqpool.tile([P, 2, 4, NCOL], _bf16)  # 8
            ps_proj = p1.enter_context(tc.tile_pool(name="ps_proj", bufs=4, space="PSUM"))

            G8A = xtqpool.tile([P, EB, 4, P], _fp8)       # 4
            G8B = xtqpool.tile([P, EB, 4, P], _bf16)      # 8
            # startup: desc-gen is ~0.6-1us serialized per queue, so split
            # the critical head across queues: G chunks on sync, xtq col 0
            # on gpsimd -- gens run in parallel; the group's DR matmuls
            # (fp8 chunks) fire first, bf16 chunks follow
            # three parallel desc-gen queues at the head: fp8 pieces on
            # sync/gpsimd, the bigger bf16 xq col on the ACT-issued queue --
            # all four startup operands land before the DR matmuls finish
            nc.sync.dma_start(G8A[:, 0, :, :], g8[:, 0, :, :])
            nc.gpsimd.dma_start(XTQ8[:, 0], xq8[:, 0])
            nc.sync.dma_start(G8B[:, 0, :, :], g16[:, 0, :, :])
            nc.scalar.dma_start(XTQ16[:, 0], xq16[:, 0])
            nc.sync.dma_start(G8A[:, 1:EB, :, :], g8[:, 1:EB, :, :])
            nc.sync.dma_start(G8B[:, 1:EB, :, :], g16[:, 1:EB, :, :])
            nc.gpsimd.dma_start(XTQ8[:, 1], xq8[:, 1])
            nc.gpsimd.dma_start(XTQ16[:, 1], xq16[:, 1])
            nc.sync.dma_start(ONES[:], onesd)
            # phase-2 data, in order of first use
            nc.sync.dma_start(XT[:, 0], xt[:, 0])
            nc.sync.dma_start(XT[:, 1], xt[:, 1])
            for i in range(2):
                nc.sync.dma_start(XK[:, 4 * i:4 * i + 4], xk[:, 4 * i:4 * i + 4])
            nc.sync.dma_start(MSK[:], msk)
            nc.sync.dma_start(XT[:, 2], xt[:, 2])
            nc.sync.dma_start(XT[:, 3], xt[:, 3])
            nc.sync.dma_start(WV[:], wv)
            for i in range(2, 4):
                nc.sync.dma_start(XK[:, 4 * i:4 * i + 4], xk[:, 4 * i:4 * i + 4])

            for jc in range(2):
                for eb in range(EB):
                    ps = ps_proj.tile([P, NCOL], _f32)
                    for i in range(2):
                        nc.tensor.matmul(ps[:], G8A[:, eb, 2 * i:2 * i + 2, :],
                                         XTQ8[:, jc, 2 * i:2 * i + 2, :],
                                         start=(i == 0), stop=False,
                                         perf_mode=_DR)
                    for k in range(4):
                        nc.tensor.matmul(ps[:], G8B[:, eb, k, :],
                                         XTQ16[:, jc, k, :],
                                         start=False, stop=(k == 3))
                    # alternate ACT/DVE so neither engine bottlenecks the copy
                    if eb % 2 == 0:
                        nc.scalar.copy(TQs[jc][:, eb, :], ps[:])
                    else:
                        nc.vector.tensor_copy(TQs[jc][:, eb, :], ps[:])


        # ---- phase 2: attention, per 256-wide local q col ----
        with ExitStack() as p2:
            ps_sc = p2.enter_context(tc.tile_pool(name="ps_sc", bufs=3, space="PSUM"))
            ps_rs = p2.enter_context(tc.tile_pool(name="ps_rs", bufs=1, space="PSUM"))
            ps_tt = p2.enter_context(tc.tile_pool(name="ps_tt", bufs=2, space="PSUM"))
            ps_out = p2.enter_context(tc.tile_pool(name="ps_out", bufs=2, space="PSUM"))
            spool = p2.enter_context(tc.tile_pool(name="spool", bufs=2))
            dpool = p2.enter_context(tc.tile_pool(name="dram", bufs=4, space="DRAM"))
            opool = p2.enter_context(tc.tile_pool(name="opool", bufs=3))

            ev = EXPS.rearrange("p k n -> p (k n)")
            # explicit EXPS slot bases give every col fresh slots while the
            # NEXT col's scores run before this col's TT (software pipeline:
            # sc0 rs0 sc1 rs1 TT0 out0 sc2 rs2 TT1 out1 sc3 rs3 TT2 out2
            # TT3 out3) -- each rowsum's DRAM roundtrip gets a whole TT+out
            # of cover, including the last col
            BASE = (0, 16, 0, 12)
            rcps = [None] * 4

            def sc_phase(jc):
                Kb = 4 * EXT[jc]
                B = BASE[jc]
                qv = TQs[jc // 2]
                qs = (jc % 2) * QW
                for kb in range(Kb):
                    # last kn block is fully masked for the first 128
                    # queries of the col: compute scores only for the
                    # upper half, zero the lower half
                    half = kb == Kb - 1
                    lo = P if half else 0
                    ps = ps_sc.tile([P, QW], _f32)
                    ic, off = kb // 4, (kb % 4) * P
                    for e2 in range(EB // 2):
                        nc.tensor.matmul(ps[:, lo:QW],
                                         XT[:, ic, 2 * e2:2 * e2 + 2, off:off + P],
                                         qv[:, 2 * e2:2 * e2 + 2, qs + lo:qs + QW],
                                         start=(e2 == 0), stop=(e2 == EB // 2 - 1),
                                         perf_mode=_DR)
                    if half:
                        nc.vector.memset(EXPS[:, B + kb, 0:P], 0.0)
                    nc.scalar.activation(EXPS[:, B + kb, lo:QW], ps[:, lo:QW],
                                         mybir.ActivationFunctionType.Exp,
                                         scale=scale)
                    if kb >= Kb - 4:
                        nc.vector.tensor_mul(EXPS[:, B + kb, lo:QW],
                                             EXPS[:, B + kb, lo:QW],
                                             MSK[:, jc * 4 + kb - (Kb - 4), lo:QW])
                # rowsums: adjacent EXPS slots pair into FD512 matmuls; the
                # pair-halves add folds into the DRAM-roundtrip readback
                rs = ps_rs.tile([1, NCOL], _f32)
                for i in range(Kb // 2):
                    nc.tensor.matmul(rs[0:1, :], ONES[:],
                                     ev[:, (B + 2 * i) * QW:(B + 2 * i + 2) * QW],
                                     start=(i == 0), stop=(i == Kb // 2 - 1))
                rs1 = spool.tile([1, NCOL], _f32, tag="rs1")
                nc.scalar.copy(rs1[0:1, :], rs[0:1, :])
                rsd = dpool.tile([1, NCOL], _f32)
                nc.sync.dma_start(rsd[:], rs1[0:1, :])
                rst = spool.tile([P, 4], _f32, tag="rst")
                nc.sync.dma_start(
                    rst[:], rsd.rearrange("o (h q p) -> (o p) (h q)", p=P, q=2))
                rsum = spool.tile([P, 2], _f32, tag="rsum")
                nc.vector.tensor_add(rsum[:], rst[:, 0:2], rst[:, 2:4])
                rcp = spool.tile([P, 2], _f32, tag=f"rcp{jc}")
                nc.vector.reciprocal(rcp[:], rsum[:])
                rcps[jc] = rcp

            def ttout_phase(jc):
                Kb = 4 * EXT[jc]
                B = BASE[jc]
                qs = jc * QW
                rcp = rcps[jc]
                # TT[d, qn] = sum_kn x[kn, d] * expT[kn, qn]  (XK resident);
                # the half-masked last block accumulates only its live half
                # (mid-group, group closed by a full-width matmul)
                kbo = list(range(Kb - 2)) + [Kb - 1, Kb - 2]
                for db in range(DB):
                    pst = ps_tt.tile([P, QW], _f32)
                    for j, kb in enumerate(kbo):
                        half = kb == Kb - 1
                        lo = P if half else 0
                        nc.tensor.matmul(pst[:, lo:QW], XK[:, kb, db, :],
                                         EXPS[:, B + kb, lo:QW],
                                         start=(j == 0), stop=(j == Kb - 1),
                                         skip_group_check=half)
                    nc.vector.tensor_copy(TTs[:, db, :], pst[:])
                # out[qn, e] = sum_d TT[d, qn] * Wv[d, e]; normalize; store
                for qb in range(2):
                    for ec in range(2):
                        po = ps_out.tile([P, NCOL], _f32)
                        for db in range(DB):
                            nc.tensor.matmul(po[:], TTs[:, db, qb * P:(qb + 1) * P],
                                             WV[:, db, ec, :],
                                             start=(db == 0), stop=(db == DB - 1))
                        ot = opool.tile([P, NCOL], _f32, tag="ot")
                        last = jc == 3 and qb == 1 and ec == 1
                        # alternate queues so desc-gen of consecutive output
                        # stores runs in parallel; the very last store goes
                        # out in halves to shorten the post-matmul tail
                        rows = out[qs + qb * P: qs + (qb + 1) * P,
                                   ec * NCOL:(ec + 1) * NCOL]
                        if last:
                            for hh, eng in ((0, nc.sync), (1, nc.gpsimd)):
                                nc.vector.tensor_scalar_mul(
                                    ot[:, hh * QW:(hh + 1) * QW],
                                    po[:, hh * QW:(hh + 1) * QW],
                                    rcp[:, qb:qb + 1])
                                eng.dma_start(rows[:, hh * QW:(hh + 1) * QW],
                                              ot[:, hh * QW:(hh + 1) * QW])
                        else:
                            nc.vector.tensor_scalar_mul(ot[:], po[:],
                                                        rcp[:, qb:qb + 1])
                            eng = nc.gpsimd if (qb + ec) % 2 else nc.sync
                            eng.dma_start(rows, ot[:])

            sc_phase(0)
            sc_phase(1)
            for jc in range(4):
                ttout_phase(jc)
                if jc + 2 < 4:
                    sc_phase(jc + 2)

    nc.compile()
    _BUILD_CACHE["nc"] = nc
    return nc


def _host_inputs(x, Wq, Wk, Wv):
    bf16 = ml_dtypes.bfloat16
    fp8 = ml_dtypes.float8_e4m3
    G = (np.asarray(Wq, np.float64) @ np.asarray(Wk, np.float64).T).astype(np.float32)
    Gt = (G * 8.0).reshape(DB, P, EB, P).transpose(1, 2, 0, 3)  # [P, EB, DB, P]
    g8_h = np.ascontiguousarray(Gt[:, :, 0:4]).astype(fp8)
    g16_h = np.ascontiguousarray(Gt[:, :, 4:8]).astype(bf16)
    wv2 = np.ascontiguousarray(
        np.asarray(Wv, np.float32).reshape(DB, P, 2, NCOL).transpose(1, 0, 2, 3)
    ).astype(bf16)
    in_maps = []
    for c in range(8):
        b, h = c // 2, c % 2
        gs = QCOLS[h]
        xb = np.asarray(x[b], dtype=np.float32)
        xbt = xb.T  # [d, n]
        xt_h = np.ascontiguousarray(
            xbt.reshape(DB, P, 4, NCOL).transpose(1, 2, 0, 3)).astype(fp8)
        qrows = np.concatenate([np.arange(gq * QW, (gq + 1) * QW) for gq in gs])
        xtq_t = xb[qrows].T.reshape(DB, P, 2, NCOL).transpose(1, 2, 0, 3)
        xq8_h = np.ascontiguousarray(xtq_t[:, :, 0:4]).astype(fp8)
        xq16_h = np.ascontiguousarray(xtq_t[:, :, 4:8]).astype(bf16)
        xk_h = np.ascontiguousarray(
            xb.reshape(16, P, DB, P).transpose(1, 0, 2, 3)).astype(bf16)
        p = np.arange(P)[:, None]
        f = np.arange(QW)[None, :]
        m = np.empty((16, P, QW), dtype=np.float32)
        for jc, gq in enumerate(gs):
            Kb = 4 * EXT[jc]
            for i, kb in enumerate(range(Kb - 4, Kb)):
                m[jc * 4 + i] = ((kb * P + p) <= (gq * QW + f)).astype(np.float32)
        in_maps.append({
            "xt": xt_h, "xq8": xq8_h, "xq16": xq16_h, "xk": xk_h,
            "g8": g8_h, "g16": g16_h, "wv": wv2,
            "msk": np.ascontiguousarray(m.transpose(1, 0, 2)).astype(bf16),
            "ones": np.ones((P, 1), bf16),
        })
    return in_maps


def kernel(x, Wq, Wk, Wv, _trace=False, _trace_kwargs=None):
    x = np.asarray(x, dtype=np.float32)
    nc = _build()
    in_maps = _host_inputs(x, Wq, Wk, Wv)
    kw = {}
    if _trace:
        kw = {"trace": True, **(_trace_kwargs or {})}
    res = run_bass_kernel_spmd(nc, in_maps, core_ids=list(range(8)), **kw)
    full = np.empty((4, NSEQ, D), dtype=np.float32)
    for c in range(8):
        b, h = c // 2, c % 2
        o = res.results[c]["out"]
        for jc, gq in enumerate(QCOLS[h]):
            full[b, gq * QW:(gq + 1) * QW] = o[jc * QW:(jc + 1) * QW]
    kernel._last_results = res
    return full


# revision 64
# speedup vs baseline: 1.1685x; 1.1685x over previous
"""Causal single-head attention on 8 trn2 NeuronCores.

Problem: x [4, 2048, 1024] f32; Wq/Wk/Wv [1024, 1024] f32.
  q,k,v = x@W*; scores = q@k^T (causal masked, scaled 1/sqrt(1024));
  out = softmax(scores) @ v.

Key algebra: scores = (x@Wq)(x@Wk)^T = x @ G @ x^T with G = Wq@Wk^T
precomputed on host. This removes the Q AND K projections from the
device: one GEMM t = x_q @ G replaces both, and the scores stationary
operand becomes raw x^T (resident in SBUF anyway). V is never built
either: out = ((attn @ x) @ Wv) / rowsum.

Sharding: 8 cores = 4 batches x 2 query-parities. Core c: batch c//2,
parity h=c%2 owns the 256-row query cols {0,3,4,7} (h=0) or {1,2,5,6}
(h=1) -- both parities see causal extents {1,2,3,4} (in 512-key cols),
so one SPMD program fits all cores; per-core causal masks ride in as
data and cover the <=256 keys of block padding per col.

Precision (rel_inf 1.24e-2 / rel_L2 1.76e-2 vs 2e-2 gate, host-simulated
exactly and bit-deterministic on hw):
  The scores matmuls run fp8 e4m3 with DoubleRow perf mode (256-deep
  contraction per instruction, 2x PE throughput): x^T (stationary) and
  t (moving) quantize to fp8 at the PSUM->SBUF copy. The t-projection
  runs HALF its contraction (d-chunks 0-3) in fp8-DR, half in bf16 --
  the error-vs-speed knee that keeps both error norms under the gate.
  The whole v path (EXPS, x, Wv) stays bf16 -- fp8 there pushes rel_L2
  past the gate. Matmul moving rate is dtype-flat (~0.5 ns/col
  measured), so fp8 pays off exactly where DoubleRow halves
  instruction count.

Schedule notes (per trace analysis): masks/x/Wv all SBUF-resident, bulk
prefetch paced on the sync DMA queue, latency-critical small DMAs on the
gpsimd queue (desc-gen ~0.6-1us serialized per queue); scores for the
next query col run before this col's TT/out so every rowsum DRAM
roundtrip hides under a full TT+out block; the fully-masked lower-half
of each col's last key block is skipped with half-width matmuls.

Per-core kernel:
  phase 1:  tT[e,qn] = G-chunks^T . xTq      (bf16, 16 groups x 8)
  phase 2, per local query col (256 wide):
    scoresT[kn,qn] = xT-chunks^T . tT        (fp8 DR, 4 matmuls/block)
    expT = exp(scoresT/32) -> bf16  (ACT; no max-sub: |s|/32 < ~3)
    last-4 kn-block tiles *= mask            (host-provided, DVE)
    rowsum[1,qn] = ones^T . expT             (FD512 pairs, bf16)
      -> DRAM roundtrip transpose -> [qn,1] -> reciprocal (off crit path)
    TT[d,qn]   = x-chunks^T . expT           (XK resident bf16)
    out[qn,e]  = TT-chunks^T . Wv            (bf16)
    out *= 1/rowsum (per-partition scalar), DMA out f32.

kernel() is self-contained: shards on host, runs via run_bass_kernel_spmd
on cores 0-7, reassembles the full [4, 2048, 1024] f32 output.
"""

import numpy as np
import ml_dtypes
from contextlib import ExitStack

import concourse.bass as bass
import concourse.mybir as mybir
import concourse.tile as tile
from concourse import bacc
from concourse.bass_utils import run_bass_kernel_spmd

P = 128
D = 1024          # d_in == d_out
NSEQ = 2048
NCOL = 512        # projection moving width / key-col unit
QW = 256          # query col width in phase 2
DB = D // P       # 8 d blocks
EB = D // P       # 8 e blocks
# local col order (2,4,3,1) by extent: tiny col ends the kernel (short tail)
EXT = (2, 4, 3, 1)           # causal extent per local q col, in 512-key cols
QCOLS = {0: (3, 7, 4, 0), 1: (2, 6, 5, 1)}  # parity -> global 256-q-cols

_f32 = mybir.dt.float32
_bf16 = mybir.dt.bfloat16
_fp8 = mybir.dt.float8e4
_DR = mybir.MatmulPerfMode.DoubleRow

_BUILD_CACHE = {}


def _build():
    if "nc" in _BUILD_CACHE:
        return _BUILD_CACHE["nc"]

    nc = bacc.Bacc("TRN2", target_bir_lowering=False, debug=False, num_devices=8)
    # host-pretiled inputs; every DMA below is contiguous per partition
    # xt[p, ic, db, n]   = x^T[db*128+p, ic*512+n]        (fp8, scores stat.)
    # xtq[p, jc, db, n]  = gathered-q x^T[db*128+p, jc*512+n]  (bf16)
    # xk[p, kb, db, m]   = x[kb*128+p, db*128+m]          (bf16, TT stat.)
    # g[p, eb, db, m]    = G[db*128+p, eb*128+m]          (bf16)
    # wv[p, db, ec, n]   = Wv[db*128+p, ec*512+n]         (bf16)
    xt = nc.dram_tensor("xt", [P, 4, DB, NCOL], _fp8, kind="ExternalInput").ap()
    # phase-1 contraction split: d-chunks 0-3 in fp8 (DoubleRow), 4-7 bf16
    xq8 = nc.dram_tensor("xq8", [P, 2, 4, NCOL], _fp8, kind="ExternalInput").ap()
    xq16 = nc.dram_tensor("xq16", [P, 2, 4, NCOL], _bf16, kind="ExternalInput").ap()
    xk = nc.dram_tensor("xk", [P, 16, DB, P], _bf16, kind="ExternalInput").ap()
    g8 = nc.dram_tensor("g8", [P, EB, 4, P], _fp8, kind="ExternalInput").ap()
    g16 = nc.dram_tensor("g16", [P, EB, 4, P], _bf16, kind="ExternalInput").ap()
    wv = nc.dram_tensor("wv", [P, DB, 2, NCOL], _bf16, kind="ExternalInput").ap()
    msk = nc.dram_tensor("msk", [P, 16, QW], _bf16, kind="ExternalInput").ap()
    onesd = nc.dram_tensor("ones", [P, 1], _bf16, kind="ExternalInput").ap()
    out = nc.dram_tensor("out", [1024, D], _f32, kind="ExternalOutput").ap()

    # G is host-prescaled by 8 (keeps the fp8 half out of e4m3 subnormals);
    # scores arrive 8x hot, folded into the exp scale
    scale = float(1.0 / np.sqrt(D) / 8.0)

    with tile.TileContext(nc) as tc, ExitStack() as ctx:
        pers = ctx.enter_context(tc.tile_pool(name="pers", bufs=1))
        XT = pers.tile([P, 4, DB, NCOL], _fp8)       # 16 KB/part
        XK = pers.tile([P, 16, DB, P], _bf16)        # 32
        # one tT tile per query-col-pair: scores for cols 0/1 then only
        # depend on the first half of phase 1 (no whole-tile WAR stall)
        TQs = [pers.tile([P, EB, NCOL], _fp8, name=f"tq{j}") for j in range(2)]
        WV = pers.tile([P, DB, 2, NCOL], _bf16)      # 16
        EXPS = pers.tile([P, 32, QW], _bf16)         # 16
        TTs = pers.tile([P, DB, QW], _bf16)          # 4
        MSK = pers.tile([P, 16, QW], _bf16)          # 8
        ONES = pers.tile([P, 1], _bf16)

        # ---- phase 1: tT projection (t = x_q @ 8G), half fp8-DR half bf16 ----
        with ExitStack() as p1:
            xtqpool = p1.enter_context(tc.tile_pool(name="xtqp", bufs=1))
            XTQ8 = xtqpool.tile([P, 2, 4, NCOL], _fp8)    # 4
            XTQ16 = xtqpool.tile([P, 2, 4, NCOL], _bf16)  # 8
            ps_proj = p1.enter_context(tc.tile_pool(name="ps_proj", bufs=4, space="PSUM"))

            G8A = xtqpool.tile([P, EB, 4, P], _fp8)       # 4
            G8B = xtqpool.tile([P, EB, 4, P], _bf16)      # 8
            # startup: desc-gen is ~0.6-1us serialized per queue, so split
            # the critical head across queues: G chunks on sync, xtq col 0
            # on gpsimd -- gens run in parallel; the group's DR matmuls
            # (fp8 chunks) fire first, bf16 chunks follow
            nc.sync.dma_start(G8A[:, 0, :, :], g8[:, 0, :, :])
            nc.gpsimd.dma_start(XTQ8[:, 0], xq8[:, 0])
            nc.sync.dma_start(G8B[:, 0, :, :], g16[:, 0, :, :])
            nc.gpsimd.dma_start(XTQ16[:, 0], xq16[:, 0])
            nc.sync.dma_start(G8A[:, 1:EB, :, :], g8[:, 1:EB, :, :])
            nc.sync.dma_start(G8B[:, 1:EB, :, :], g16[:, 1:EB, :, :])
            nc.gpsimd.dma_start(XTQ8[:, 1], xq8[:, 1])
            nc.gpsimd.dma_start(XTQ16[:, 1], xq16[:, 1])
            nc.sync.dma_start(ONES[:], onesd)
            # phase-2 data, in order of first use
            nc.sync.dma_start(XT[:, 0], xt[:, 0])
            nc.sync.dma_start(XT[:, 1], xt[:, 1])
            for i in range(2):
                nc.sync.dma_start(XK[:, 4 * i:4 * i + 4], xk[:, 4 * i:4 * i + 4])
            nc.sync.dma_start(MSK[:], msk)
            nc.sync.dma_start(XT[:, 2], xt[:, 2])
            nc.sync.dma_start(XT[:, 3], xt[:, 3])
            nc.sync.dma_start(WV[:], wv)
            for i in range(2, 4):
                nc.sync.dma_start(XK[:, 4 * i:4 * i + 4], xk[:, 4 * i:4 * i + 4])

            for jc in range(2):
                for eb in range(EB):
                    ps = ps_proj.tile([P, NCOL], _f32)
                    for i in range(2):
                        nc.tensor.matmul(ps[:], G8A[:, eb, 2 * i:2 * i + 2, :],
                                         XTQ8[:, jc, 2 * i:2 * i + 2, :],
                                         start=(i == 0), stop=False,
                                         perf_mode=_DR)
                    for k in range(4):
                        nc.tensor.matmul(ps[:], G8B[:, eb, k, :],
                                         XTQ16[:, jc, k, :],
                                         start=False, stop=(k == 3))
                    # alternate ACT/DVE so neither engine bottlenecks the copy
                    if eb % 2 == 0:
                        nc.scalar.copy(TQs[jc][:, eb, :], ps[:])
                    else:
                        nc.vector.tensor_copy(TQs[jc][:, eb, :], ps[:])


        # ---- phase 2: attention, per 256-wide local q col ----
        with ExitStack() as p2:
            ps_sc = p2.enter_context(tc.tile_pool(name="ps_sc", bufs=3, space="PSUM"))
            ps_rs = p2.enter_context(tc.tile_pool(name="ps_rs", bufs=1, space="PSUM"))
            ps_tt = p2.enter_context(tc.tile_pool(name="ps_tt", bufs=2, space="PSUM"))
            ps_out = p2.enter_context(tc.tile_pool(name="ps_out", bufs=2, space="PSUM"))
            spool = p2.enter_context(tc.tile_pool(name="spool", bufs=2))
            dpool = p2.enter_context(tc.tile_pool(name="dram", bufs=4, space="DRAM"))
            opool = p2.enter_context(tc.tile_pool(name="opool", bufs=3))

            ev = EXPS.rearrange("p k n -> p (k n)")
            # explicit EXPS slot bases give every col fresh slots while the
            # NEXT col's scores run before this col's TT (software pipeline:
            # sc0 rs0 sc1 rs1 TT0 out0 sc2 rs2 TT1 out1 sc3 rs3 TT2 out2
            # TT3 out3) -- each rowsum's DRAM roundtrip gets a whole TT+out
            # of cover, including the last col
            BASE = (0, 16, 0, 12)
            rcps = [None] * 4

            def sc_phase(jc):
                Kb = 4 * EXT[jc]
                B = BASE[jc]
                qv = TQs[jc // 2]
                qs = (jc % 2) * QW
                for kb in range(Kb):
                    # last kn block is fully masked for the first 128
                    # queries of the col: compute scores only for the
                    # upper half, zero the lower half
                    half = kb == Kb - 1
                    lo = P if half else 0
                    ps = ps_sc.tile([P, QW], _f32)
                    ic, off = kb // 4, (kb % 4) * P
                    for e2 in range(EB // 2):
                        nc.tensor.matmul(ps[:, lo:QW],
                                         XT[:, ic, 2 * e2:2 * e2 + 2, off:off + P],
                                         qv[:, 2 * e2:2 * e2 + 2, qs + lo:qs + QW],
                                         start=(e2 == 0), stop=(e2 == EB // 2 - 1),
                                         perf_mode=_DR)
                    if half:
                        nc.vector.memset(EXPS[:, B + kb, 0:P], 0.0)
                    nc.scalar.activation(EXPS[:, B + kb, lo:QW], ps[:, lo:QW],
                                         mybir.ActivationFunctionType.Exp,
                                         scale=scale)
                    if kb >= Kb - 4:
                        nc.vector.tensor_mul(EXPS[:, B + kb, lo:QW],
                                             EXPS[:, B + kb, lo:QW],
                                             MSK[:, jc * 4 + kb - (Kb - 4), lo:QW])
                # rowsums: adjacent EXPS slots pair into FD512 matmuls; the
                # pair-halves add folds into the DRAM-roundtrip readback
                rs = ps_rs.tile([1, NCOL], _f32)
                for i in range(Kb // 2):
                    nc.tensor.matmul(rs[0:1, :], ONES[:],
                                     ev[:, (B + 2 * i) * QW:(B + 2 * i + 2) * QW],
                                     start=(i == 0), stop=(i == Kb // 2 - 1))
                rs1 = spool.tile([1, NCOL], _f32, tag="rs1")
                nc.scalar.copy(rs1[0:1, :], rs[0:1, :])
                rsd = dpool.tile([1, NCOL], _f32)
                nc.sync.dma_start(rsd[:], rs1[0:1, :])
                rst = spool.tile([P, 4], _f32, tag="rst")
                nc.sync.dma_start(
                    rst[:], rsd.rearrange("o (h q p) -> (o p) (h q)", p=P, q=2))
                rsum = spool.tile([P, 2], _f32, tag="rsum")
                nc.vector.tensor_add(rsum[:], rst[:, 0:2], rst[:, 2:4])
                rcp = spool.tile([P, 2], _f32, tag=f"rcp{jc}")
                nc.vector.reciprocal(rcp[:], rsum[:])
                rcps[jc] = rcp

            def ttout_phase(jc):
                Kb = 4 * EXT[jc]
                B = BASE[jc]
                qs = jc * QW
                rcp = rcps[jc]
                # TT[d, qn] = sum_kn x[kn, d] * expT[kn, qn]  (XK resident);
                # the half-masked last block accumulates only its live half
                # (mid-group, group closed by a full-width matmul)
                kbo = list(range(Kb - 2)) + [Kb - 1, Kb - 2]
                for db in range(DB):
                    pst = ps_tt.tile([P, QW], _f32)
                    for j, kb in enumerate(kbo):
                        half = kb == Kb - 1
                        lo = P if half else 0
                        nc.tensor.matmul(pst[:, lo:QW], XK[:, kb, db, :],
                                         EXPS[:, B + kb, lo:QW],
                                         start=(j == 0), stop=(j == Kb - 1),
                                         skip_group_check=half)
                    nc.vector.tensor_copy(TTs[:, db, :], pst[:])
                # out[qn, e] = sum_d TT[d, qn] * Wv[d, e]; normalize; store
                for qb in range(2):
                    for ec in range(2):
                        po = ps_out.tile([P, NCOL], _f32)
                        for db in range(DB):
                            nc.tensor.matmul(po[:], TTs[:, db, qb * P:(qb + 1) * P],
                                             WV[:, db, ec, :],
                                             start=(db == 0), stop=(db == DB - 1))
                        ot = opool.tile([P, NCOL], _f32, tag="ot")
                        last = jc == 3 and qb == 1 and ec == 1
                        # alternate queues so desc-gen of consecutive output
                        # stores runs in parallel; the very last store goes
                        # out in halves to shorten the post-matmul tail
                        rows = out[qs + qb * P: qs + (qb + 1) * P,
                                   ec * NCOL:(ec + 1) * NCOL]
                        if last:
                            for hh, eng in ((0, nc.sync), (1, nc.gpsimd)):
                                nc.vector.tensor_scalar_mul(
                                    ot[:, hh * QW:(hh + 1) * QW],
                                    po[:, hh * QW:(hh + 1) * QW],
                                    rcp[:, qb:qb + 1])
                                eng.dma_start(rows[:, hh * QW:(hh + 1) * QW],
                                              ot[:, hh * QW:(hh + 1) * QW])
                        else:
                            nc.vector.tensor_scalar_mul(ot[:], po[:],
                                                        rcp[:, qb:qb + 1])
                            eng = nc.gpsimd if (qb + ec) % 2 else nc.sync
                            eng.dma_start(rows, ot[:])

            sc_phase(0)
            sc_phase(1)
            for jc in range(4):
                ttout_phase(jc)
                if jc + 2 < 4:
                    sc_phase(jc + 2)

    nc.compile()
    _BUILD_CACHE["nc"] = nc
    return nc


def _host_inputs(x, Wq, Wk, Wv):
    bf16 = ml_dtypes.bfloat16
    fp8 = ml_dtypes.float8_e4m3
    G = (np.asarray(Wq, np.float64) @ np.asarray(Wk, np.float64).T).astype(np.float32)
    Gt = (G * 8.0).reshape(DB, P, EB, P).transpose(1, 2, 0, 3)  # [P, EB, DB, P]
    g8_h = np.ascontiguousarray(Gt[:, :, 0:4]).astype(fp8)
    g16_h = np.ascontiguousarray(Gt[:, :, 4:8]).astype(bf16)
    wv2 = np.ascontiguousarray(
        np.asarray(Wv, np.float32).reshape(DB, P, 2, NCOL).transpose(1, 0, 2, 3)
    ).astype(bf16)
    in_maps = []
    for c in range(8):
        b, h = c // 2, c % 2
        gs = QCOLS[h]
        xb = np.asarray(x[b], dtype=np.float32)
        xbt = xb.T  # [d, n]
        xt_h = np.ascontiguousarray(
            xbt.reshape(DB, P, 4, NCOL).transpose(1, 2, 0, 3)).astype(fp8)
        qrows = np.concatenate([np.arange(gq * QW, (gq + 1) * QW) for gq in gs])
        xtq_t = xb[qrows].T.reshape(DB, P, 2, NCOL).transpose(1, 2, 0, 3)
        xq8_h = np.ascontiguousarray(xtq_t[:, :, 0:4]).astype(fp8)
        xq16_h = np.ascontiguousarray(xtq_t[:, :, 4:8]).astype(bf16)
        xk_h = np.ascontiguousarray(
            xb.reshape(16, P, DB, P).transpose(1, 0, 2, 3)).astype(bf16)
        p = np.arange(P)[:, None]
        f = np.arange(QW)[None, :]
        m = np.empty((16, P, QW), dtype=np.float32)
        for jc, gq in enumerate(gs):
            Kb = 4 * EXT[jc]
            for i, kb in enumerate(range(Kb - 4, Kb)):
                m[jc * 4 + i] = ((kb * P + p) <= (gq * QW + f)).astype(np.float32)
        in_maps.append({
            "xt": xt_h, "xq8": xq8_h, "xq16": xq16_h, "xk": xk_h,
            "g8": g8_h, "g16": g16_h, "wv": wv2,
            "msk": np.ascontiguousarray(m.transpose(1, 0, 2)).astype(bf16),
            "ones": np.ones((P, 1), bf16),
        })
    return in_maps


def kernel(x, Wq, Wk, Wv, _trace=False, _trace_kwargs=None):
    x = np.asarray(x, dtype=np.float32)
    nc = _build()
    in_maps = _host_inputs(x, Wq, Wk, Wv)
    kw = {}
    if _trace:
        kw = {"trace": True, **(_trace_kwargs or {})}
    res = run_bass_kernel_spmd(nc, in_maps, core_ids=list(range(8)), **kw)
    full = np.empty((4, NSEQ, D), dtype=np.float32)
    for c in range(8):
        b, h = c // 2, c % 2
        o = res.results[c]["out"]
        for jc, gq in enumerate(QCOLS[h]):
            full[b, gq * QW:(gq + 1) * QW] = o[jc * QW:(jc + 1) * QW]
    kernel._last_results = res
    return full
